# revision 1
# baseline (speedup 1.0000x reference)
"""Bass/Trainium2 kernel for nn_BakaAttention: 8-way data-parallel over batch.

Per core (one batch element):
  q = rope(x@wq, off=1024); k = rope(concat(past_k, x@wk), off=0); v = concat(past_v, x@wv)
  out = softmax(mask(q k^T / 16)) v @ wo

Layouts on chip: qT/kT are feature-major [f, t]; scores computed transposed
[s, t] so PV consumes probs directly as the stationary operand; softmax
row-sums ride along as a 257th "ones" column of the moving v operand.
All matmuls run in float32r (full PE rate at N>=256, ~1e-4 rel err).
"""

import numpy as np

B, T, P, H, DH, DIN, DOUT = 8, 1024, 1024, 4, 256, 1024, 1152
S = P + T  # 2048 keys
THETA = 10000.0
NCORES = 8


def _host_constants():
    m = np.arange(0, DH, 2, dtype=np.float64) / DH          # 128 freqs
    inv = 1.0 / (THETA ** m)                                # [128]
    pos = np.arange(S, dtype=np.float64)                    # [2048]
    ang = np.outer(inv, pos)                                # [128, 2048]
    cos_full = np.cos(ang)
    sin_full = np.sin(ang)
    r = np.arange(128) // 2
    consts = {
        "cos_lo": cos_full[r, :].astype(np.float32),
        "cos_hi": cos_full[64 + r, :].astype(np.float32),
        "sin_lo": sin_full[r, :].astype(np.float32),
        "sin_hi": sin_full[64 + r, :].astype(np.float32),
    }
    prot = np.zeros((128, 128), np.float32)
    for mm in range(64):
        prot[2 * mm, 2 * mm + 1] = 1.0
        prot[2 * mm + 1, 2 * mm] = -1.0
    consts["prot"] = prot
    consts["ident"] = np.eye(128, dtype=np.float32)
    # masks[ci][sl, tl] = 1.0 if sl <= tl - 128*ci else 0 (keep), ci in 0..3
    sl = np.arange(128)[:, None]
    tl = np.arange(512)[None, :]
    masks = np.stack(
        [(sl <= tl - 128 * ci).astype(np.float32) for ci in range(4)], axis=1
    )  # [128, 4, 512]
    consts["masks"] = np.ascontiguousarray(masks)
    consts["ones"] = np.ones((128, 4), np.float32)
    consts["onesr"] = np.ones((1, 128), np.float32)
    return consts


def build_kernel(debug=False):
    import concourse.bass as bass
    import concourse.mybir as mybir
    from concourse import bacc
    from concourse.tile import TileContext

    f32 = mybir.dt.float32
    f32r = mybir.dt.float32r
    AF = mybir.ActivationFunctionType
    OP = mybir.AluOpType

    nc = bacc.Bacc(None, target_bir_lowering=False)

    x_d = nc.dram_tensor("x", [T, DIN], f32r, kind="ExternalInput")
    pk_d = nc.dram_tensor("past_k", [P, H, DH], f32r, kind="ExternalInput")
    pv_d = nc.dram_tensor("past_v", [P, H, DH], f32r, kind="ExternalInput")
    wq_d = nc.dram_tensor("wq", [DIN, DIN], f32r, kind="ExternalInput")
    wk_d = nc.dram_tensor("wk", [DIN, DIN], f32r, kind="ExternalInput")
    wv_d = nc.dram_tensor("wv", [DIN, DIN], f32r, kind="ExternalInput")
    wo_d = nc.dram_tensor("wo", [DIN, DOUT], f32r, kind="ExternalInput")
    cos_lo_d = nc.dram_tensor("cos_lo", [128, S], f32, kind="ExternalInput")
    cos_hi_d = nc.dram_tensor("cos_hi", [128, S], f32, kind="ExternalInput")
    sin_lo_d = nc.dram_tensor("sin_lo", [128, S], f32, kind="ExternalInput")
    sin_hi_d = nc.dram_tensor("sin_hi", [128, S], f32, kind="ExternalInput")
    prot_d = nc.dram_tensor("prot", [128, 128], f32r, kind="ExternalInput")
    ident_d = nc.dram_tensor("ident", [128, 128], f32r, kind="ExternalInput")
    masks_d = nc.dram_tensor("masks", [128, 4, 512], f32, kind="ExternalInput")
    ones_d = nc.dram_tensor("ones", [128, 4], f32r, kind="ExternalInput")
    onesr_d = nc.dram_tensor("onesr", [1, 128], f32r, kind="ExternalInput")
    out_d = nc.dram_tensor("out", [T, DOUT], f32, kind="ExternalOutput")
    vkind = dict(kind="ExternalOutput") if debug else {}
    v_r = nc.dram_tensor("v_r", [T, DIN], f32r, **vkind)
    qT_r = nc.dram_tensor("qT_r", [8, 128, T], f32r, **vkind)
    if debug:
        kT_dump = nc.dram_tensor("kT_dump", [8, 128, S], f32r, kind="ExternalOutput")
        y_dump = nc.dram_tensor("y_dump", [8, 128, DIN], f32, kind="ExternalOutput")

    from contextlib import ExitStack
    stack = ExitStack()
    with TileContext(nc) as tc, stack:
        cstp = stack.enter_context(tc.tile_pool(name="consts", bufs=1))
        prot = cstp.tile([128, 128], f32r, name="prot", tag="prot")
        ident = cstp.tile([128, 128], f32r, name="ident", tag="ident")
        masks = cstp.tile([128, 4, 512], f32, name="masks", tag="masks")
        ones_sb = cstp.tile([128, 4], f32r, name="ones_sb", tag="ones_sb")
        nc.sync.dma_start(out=ones_sb[:], in_=ones_d[:])
        onesr_sb = cstp.tile([1, 128], f32r, name="onesr_sb", tag="onesr_sb")
        nc.sync.dma_start(out=onesr_sb[:], in_=onesr_d[:])
        nc.sync.dma_start(out=prot[:], in_=prot_d[:])
        nc.sync.dma_start(out=ident[:], in_=ident_d[:])
        nc.sync.dma_start(out=masks[:], in_=masks_d[:])

        resid = stack.enter_context(tc.tile_pool(name="resid", bufs=1))
        kT = [resid.tile([128, S], f32r, name=f"kT{i}", tag=f"kT{i}") for i in range(8)]

        # ---------------- Phase 1+2: xT, projections, rope ----------------
        with tc.tile_pool(name="tables", bufs=1) as tabp, \
             tc.tile_pool(name="p2xT", bufs=1) as xtp, \
             tc.tile_pool(name="p2", bufs=2) as p2p, \
             tc.tile_pool(name="p2st", bufs=3) as stp, \
             tc.tile_pool(name="p2ps", bufs=4, space="PSUM") as ps2, \
             tc.tile_pool(name="p2rot", bufs=2, space="PSUM") as rotps, \
             tc.tile_pool(name="p2kp", bufs=1) as kpp:
            cos_t = [tabp.tile([128, T], f32, name="clo", tag="clo"),
                     tabp.tile([128, T], f32, name="chi", tag="chi")]
            sin_t = [tabp.tile([128, T], f32, name="slo", tag="slo"),
                     tabp.tile([128, T], f32, name="shi", tag="shi")]

            def load_tables(p0):
                nc.sync.dma_start(out=cos_t[0][:], in_=cos_lo_d[:, p0:p0 + T])
                nc.sync.dma_start(out=cos_t[1][:], in_=cos_hi_d[:, p0:p0 + T])
                nc.sync.dma_start(out=sin_t[0][:], in_=sin_lo_d[:, p0:p0 + T])
                nc.sync.dma_start(out=sin_t[1][:], in_=sin_hi_d[:, p0:p0 + T])

            load_tables(P)  # positions 1024..2047 for q and new-k

            xT = [xtp.tile([128, T], f32r, name=f"xT{i}", tag=f"xT{i}") for i in range(8)]
            for tt in range(8):
                xt = p2p.tile([128, DIN], f32r, name="xload", tag="xload")
                nc.sync.dma_start(out=xt[:], in_=x_d[128 * tt:128 * (tt + 1), :])
                for kt in range(8):
                    tp = ps2.tile([128, 128], f32, name="tps", tag="tps", bufs=2)
                    nc.tensor.matmul(tp[:], xt[:, 128 * kt:128 * (kt + 1)], ident[:],
                                     start=True, stop=True)
                    nc.scalar.copy(xT[kt][:, 128 * tt:128 * (tt + 1)], tp[:])

            def rope_combine(dst_ap, raw_sb, rot_ps, ft, off, n):
                # dst = raw * cos + rot * sin ; table rows by f-tile parity
                ctab = cos_t[ft % 2][:, off:off + n]
                stab = sin_t[ft % 2][:, off:off + n]
                t1 = p2p.tile([128, 512], f32, name="ropet1", tag="ropet1")
                nc.gpsimd.tensor_tensor(t1[:, :n], raw_sb, ctab, op=OP.mult)
                t2 = p2p.tile([128, 512], f32, name="ropet2", tag="ropet2")
                nc.vector.tensor_tensor(t2[:, :n], rot_ps, stab, op=OP.mult)
                nc.vector.tensor_tensor(dst_ap, t1[:, :n], t2[:, :n], op=OP.add)

            # q and new-k projections (transposed layout) + rope
            for w_d, dst in ((wq_d, None), (wk_d, kT)):
                for ftg in range(4):          # pairs of f-tiles
                    psl = [ps2.tile([128, 512], f32, name=f"pj{i}", tag=f"pj{i}", bufs=1) for i in range(4)]
                    for kt in range(8):
                        wt = stp.tile([128, 256], f32r, name="wload", tag="wload")
                        nc.sync.dma_start(
                            out=wt[:],
                            in_=w_d[128 * kt:128 * (kt + 1), 256 * ftg:256 * (ftg + 1)])
                        for f2 in range(2):
                            for th in range(2):
                                nc.tensor.matmul(
                                    psl[2 * f2 + th][:],
                                    wt[:, 128 * f2:128 * (f2 + 1)].bitcast(f32r),
                                    xT[kt][:, 512 * th:512 * (th + 1)].bitcast(f32r),
                                    start=(kt == 0), stop=(kt == 7))
                    for f2 in range(2):
                        ft = 2 * ftg + f2
                        raw = p2p.tile([128, 1024], f32r, name="rawsb", tag="rawsb")
                        for th in range(2):
                            nc.scalar.copy(raw[:, 512 * th:512 * (th + 1)],
                                           psl[2 * f2 + th][:])
                        if dst is None:
                            qstage = p2p.tile([128, 1024], f32r, name="qstage",
                                              tag="qstage")
                        for th in range(2):
                            rp = rotps.tile([128, 512], f32, name="rotps", tag="rotps")
                            nc.tensor.matmul(rp[:], prot[:].bitcast(f32r),
                                             raw[:, 512 * th:512 * (th + 1)].bitcast(f32r),
                                             start=True, stop=True)
                            if dst is None:
                                dst_ap = qstage[:, 512 * th:512 * (th + 1)]
                            else:
                                dst_ap = dst[ft][:, P + 512 * th:P + 512 * (th + 1)]
                            rope_combine(dst_ap, raw[:, 512 * th:512 * (th + 1)],
                                         rp[:], ft, 512 * th, 512)
                        if dst is None:
                            nc.sync.dma_start(out=qT_r[ftg * 2 + f2], in_=qstage[:])

            # v projection, natural layout [s, f] -> DRAM
            for stg in range(4):
                psl = [ps2.tile([128, 512], f32, name=f"pv{i}", tag=f"pj{i}", bufs=1) for i in range(4)]
                for kt in range(8):
                    wt = stp.tile([128, 1024], f32r, name="wvload", tag="wvload")
                    nc.sync.dma_start(out=wt[:], in_=wv_d[128 * kt:128 * (kt + 1), :])
                    for s2 in range(2):
                        st = 2 * stg + s2
                        for fh in range(2):
                            nc.tensor.matmul(
                                psl[2 * s2 + fh][:],
                                xT[kt][:, 128 * st:128 * (st + 1)].bitcast(f32r),
                                wt[:, 512 * fh:512 * (fh + 1)].bitcast(f32r),
                                start=(kt == 0), stop=(kt == 7))
                for s2 in range(2):
                    st = 2 * stg + s2
                    vsb = p2p.tile([128, 1024], f32r, name="vsb", tag="vsb")
                    for fh in range(2):
                        nc.scalar.copy(vsb[:, 512 * fh:512 * (fh + 1)],
                                       psl[2 * s2 + fh][:])
                    nc.sync.dma_start(out=v_r[128 * st:128 * (st + 1), :], in_=vsb[:])

            # past_k: transpose + rope into kT[:, 0:1024]
            load_tables(0)  # positions 0..1023
            for h in range(4):
                kp = [kpp.tile([128, P], f32r, name=f"kp{i}", tag=f"kp{i}") for i in range(2)]
                for st in range(8):
                    pkt = stp.tile([128, DH], f32r, name="pkload", tag="pkload")
                    nc.sync.dma_start(out=pkt[:],
                                      in_=pk_d[128 * st:128 * (st + 1), h, :])
                    for f2 in range(2):
                        tp = ps2.tile([128, 128], f32, name="tps", tag="tps", bufs=2)
                        nc.tensor.matmul(tp[:], pkt[:, 128 * f2:128 * (f2 + 1)],
                                         ident[:], start=True, stop=True)
                        nc.scalar.copy(kp[f2][:, 128 * st:128 * (st + 1)], tp[:])
                for f2 in range(2):
                    ft = 2 * h + f2
                    for sh in range(2):
                        rp = rotps.tile([128, 512], f32, name="rotps", tag="rotps")
                        nc.tensor.matmul(rp[:], prot[:].bitcast(f32r),
                                         kp[f2][:, 512 * sh:512 * (sh + 1)].bitcast(f32r),
                                         start=True, stop=True)
                        rope_combine(kT[ft][:, 512 * sh:512 * (sh + 1)],
                                     kp[f2][:, 512 * sh:512 * (sh + 1)],
                                     rp[:], ft, 512 * sh, 512)

        if debug:
            for i in range(8):
                nc.sync.dma_start(out=kT_dump[i], in_=kT[i][:])

        # ---------------- Phase 3: attention ----------------
        ysbp = stack.enter_context(tc.tile_pool(name="ysb", bufs=1))
        yT = [ysbp.tile([128, T], f32r, name=f"yT{i}", tag=f"yT{i}")
              for i in range(8)]
        with tc.tile_pool(name="vaug", bufs=1) as vap, \
             tc.tile_pool(name="qth", bufs=2) as qtp, \
             tc.tile_pool(name="probs", bufs=5) as prp, \
             tc.tile_pool(name="p3sm", bufs=4) as smp, \
             tc.tile_pool(name="p3sc", bufs=3, space="PSUM") as scps, \
             tc.tile_pool(name="p3y", bufs=1, space="PSUM") as yps:
            for h in range(4):
                qh = [qtp.tile([128, T], f32r, name=f"qh{fk}", tag=f"qh{fk}")
                      for fk in range(2)]
                for fk in range(2):
                    nc.sync.dma_start(out=qh[fk][:], in_=qT_r[2 * h + fk])
                va = [vap.tile([128, 260], f32r, name=f"va{j}", tag=f"va{j}")
                      for j in range(16)]
                for j in range(16):
                    if j < 8:
                        src = pv_d[128 * j:128 * (j + 1), h, :]
                    else:
                        src = v_r[128 * (j - 8):128 * (j - 7),
                                  DH * h:DH * (h + 1)]
                    nc.sync.dma_start(out=va[j][:, 0:DH], in_=src)
                for TH in range(2):
                    jmax = 12 + 4 * TH
                    ytp_ps = [yps.tile([128, 512], f32, name=f"ytp{i}",
                                       tag=f"ytp{i}", bufs=1) for i in range(2)]
                    sm_ps = yps.tile([1, 512], f32, name="smps", tag="smps",
                                     bufs=1)
                    for j in range(jmax):
                        sc = scps.tile([128, 512], f32, name="sc", tag="sc")
                        for fk in range(2):
                            nc.tensor.matmul(
                                sc[:],
                                kT[2 * h + fk][:, 128 * j:128 * (j + 1)].bitcast(f32r),
                                qh[fk][:, 512 * TH:512 * (TH + 1)].bitcast(f32r),
                                start=(fk == 0), stop=(fk == 1))
                        pj = prp.tile([128, 512], f32r, name="pj", tag="pj")
                        nc.scalar.activation(pj[:], sc[:], AF.Exp, scale=float(DH ** -0.5))
                        ci = j - (8 + 4 * TH)
                        if ci >= 0:
                            nc.gpsimd.tensor_tensor(pj[:], pj[:], masks[:, ci, :],
                                                    op=OP.mult)
                        for fb in range(2):
                            nc.tensor.matmul(
                                ytp_ps[fb][:],
                                va[j][:, 128 * fb:128 * (fb + 1)],
                                pj[:],
                                start=(j == 0), stop=(j == jmax - 1))
                        nc.tensor.matmul(
                            sm_ps[:], ones_sb[:, 0:1], pj[:],
                            start=(j == 0), stop=(j == jmax - 1))
                    rc = smp.tile([1, 512], f32r, name="rc", tag="rc")
                    with nc.allow_low_precision(reason="f32r bits == f32"):
                        nc.vector.reciprocal(rc[:], sm_ps[:])
                    bc_ps = scps.tile([128, 512], f32, name="bcps", tag="bcps",
                                      bufs=1)
                    nc.tensor.matmul(bc_ps[:], onesr_sb[:], rc[:],
                                     start=True, stop=True)
                    bc_sb = smp.tile([128, 512], f32, name="bcsb", tag="bcsb")
                    nc.scalar.copy(bc_sb[:], bc_ps[:])
                    for fb in range(2):
                        nc.vector.tensor_tensor(
                            yT[2 * h + fb][:, 512 * TH:512 * (TH + 1)],
                            ytp_ps[fb][:],
                            bc_sb[:],
                            op=OP.mult)

        # ---------------- Phase 4: o-projection ----------------
        with tc.tile_pool(name="p4wo", bufs=1) as wop, \
             tc.tile_pool(name="p4o", bufs=2) as osp, \
             tc.tile_pool(name="p4ps", bufs=4, space="PSUM") as ps4:
            wo_sb = [wop.tile([128, DOUT], f32r, name=f"wo{i}", tag=f"wo{i}")
                     for i in range(8)]
            for kt in range(8):
                nc.sync.dma_start(out=wo_sb[kt][:],
                                  in_=wo_d[128 * kt:128 * (kt + 1), :])
            for tt in range(8):
                ot = osp.tile([128, DOUT], f32, name="osb", tag="osb")
                for ds in range(3):
                    op_ps = ps4.tile([128, 384], f32, name="ops", tag="ops", bufs=3)
                    for fk in range(8):
                        nc.tensor.matmul(
                            op_ps[:],
                            yT[fk][:, 128 * tt:128 * (tt + 1)],
                            wo_sb[fk][:, 384 * ds:384 * (ds + 1)],
                            start=(fk == 0), stop=(fk == 7))
                    nc.scalar.copy(ot[:, 384 * ds:384 * (ds + 1)], op_ps[:])
                nc.sync.dma_start(out=out_d[128 * tt:128 * (tt + 1), :], in_=ot[:])

    nc.finalize()
    return nc


_NC_CACHE = {}


def run(x, past_k, past_v, wq, wk, wv, wo, debug=False, trace=False):
    from concourse.bass_utils import run_bass_kernel_spmd

    key = (debug,)
    if key not in _NC_CACHE:
        _NC_CACHE[key] = build_kernel(debug=debug)
    nc = _NC_CACHE[key]
    consts = _host_constants()
    in_maps = []
    for b in range(NCORES):
        m = {
            "x": np.ascontiguousarray(x[b]),
            "past_k": np.ascontiguousarray(past_k[b]),
            "past_v": np.ascontiguousarray(past_v[b]),
            "wq": wq, "wk": wk, "wv": wv, "wo": wo,
            "cos_lo": consts["cos_lo"], "cos_hi": consts["cos_hi"],
            "sin_lo": consts["sin_lo"], "sin_hi": consts["sin_hi"],
            "prot": consts["prot"], "ident": consts["ident"],
            "masks": consts["masks"], "ones": consts["ones"], "onesr": consts["onesr"],
        }
        in_maps.append(m)
    res = run_bass_kernel_spmd(nc, in_maps, list(range(NCORES)), trace=trace)
    out = np.stack([res.results[b]["out"] for b in range(NCORES)], axis=0)
    return out, res


def kernel(x, past_k, past_v, wq, wk, wv, wo):
    out, _ = run(x, past_k, past_v, wq, wk, wv, wo)
    return out



# revision 4
# speedup vs baseline: 1.5199x; 1.5199x over previous
"""Bass/Trainium2 kernel for nn_BakaAttention: 8-way data-parallel over batch.

Per core (one batch element):
  q = rope(x@wq, off=1024); k = rope(concat(past_k, x@wk), off=0); v = concat(past_v, x@wv)
  out = softmax(mask(q k^T / 16)) v @ wo

All matmul operands are bf16 (1 cycle/row on PE, half the DMA/SBUF bytes);
PSUM accumulation stays f32. Host pre-transposes x -> xT and past_k -> f-major
tiles, and pre-permutes q/k feature order per head to [even feats | odd feats],
which turns interleaved-pair rope into partition-aligned elementwise ops
(no on-chip transposes or rotation matmuls). Scores are computed transposed
[s, t] so PV consumes probs directly as the moving operand; everything stays
SBUF-resident (no DRAM round trips).
"""

import numpy as np

B, T, P, H, DH, DIN, DOUT = 8, 1024, 1024, 4, 256, 1024, 1152
S = P + T  # 2048 keys
THETA = 10000.0
NCORES = 8


def _bf16():
    import ml_dtypes
    return ml_dtypes.bfloat16


def _host_constants():
    bf16 = _bf16()
    m = np.arange(128, dtype=np.float64)
    inv = 1.0 / (THETA ** (m / 128.0))                      # [128]
    pos = np.arange(S, dtype=np.float64)                    # [2048]
    ang = np.outer(inv, pos)                                # [128, 2048]
    consts = {
        "cos": np.cos(ang).astype(bf16),
        "sin": np.sin(ang).astype(bf16),
    }
    # masks[sl, ci, tl] = 1.0 if sl <= tl - 128*ci else 0 (keep), ci in 0..3
    sl = np.arange(128)[:, None]
    tl = np.arange(512)[None, :]
    masks = np.stack(
        [(sl <= tl - 128 * ci).astype(bf16) for ci in range(4)], axis=1
    )  # [128, 4, 512]
    consts["masks"] = np.ascontiguousarray(masks)
    consts["ones"] = np.ones((128, 1), bf16)
    consts["onesr"] = np.ones((1, 128), np.float32)
    return consts


# column permutation putting each head's even features first, then odd
def _qk_perm():
    idx = np.arange(DIN).reshape(H, DH // 2, 2)
    return np.concatenate([idx[..., 0], idx[..., 1]], axis=1).reshape(-1)


def build_kernel():
    import concourse.bass as bass
    import concourse.mybir as mybir
    from concourse import bacc
    from concourse.tile import TileContext

    f32 = mybir.dt.float32
    f32r = mybir.dt.float32r
    bf = mybir.dt.bfloat16
    AF = mybir.ActivationFunctionType
    OP = mybir.AluOpType

    nc = bacc.Bacc(None, target_bir_lowering=False)

    xT_d = nc.dram_tensor("xT", [DIN, T], bf, kind="ExternalInput")
    pkT_d = nc.dram_tensor("pkT", [8, 128, P], bf, kind="ExternalInput")
    pv_d = nc.dram_tensor("pv", [P, DIN], bf, kind="ExternalInput")
    wq_d = nc.dram_tensor("wq", [DIN, DIN], bf, kind="ExternalInput")
    wk_d = nc.dram_tensor("wk", [DIN, DIN], bf, kind="ExternalInput")
    wv_d = nc.dram_tensor("wv", [DIN, DIN], bf, kind="ExternalInput")
    wo_d = nc.dram_tensor("wo", [DIN, DOUT], bf, kind="ExternalInput")
    cos_d = nc.dram_tensor("cos", [128, S], bf, kind="ExternalInput")
    sin_d = nc.dram_tensor("sin", [128, S], bf, kind="ExternalInput")
    masks_d = nc.dram_tensor("masks", [128, 4, 512], bf, kind="ExternalInput")
    ones_d = nc.dram_tensor("ones", [128, 1], bf, kind="ExternalInput")
    onesr_d = nc.dram_tensor("onesr", [1, 128], f32r, kind="ExternalInput")
    out_d = nc.dram_tensor("out", [T, DOUT], f32, kind="ExternalOutput")

    from contextlib import ExitStack
    stack = ExitStack()
    with TileContext(nc) as tc, stack:
        cstp = stack.enter_context(tc.tile_pool(name="consts", bufs=1))
        masks = cstp.tile([128, 4, 512], bf, name="masks", tag="masks")
        ones_sb = cstp.tile([128, 1], bf, name="ones_sb", tag="ones_sb")
        onesr_sb = cstp.tile([1, 128], f32r, name="onesr_sb", tag="onesr_sb")
        cos_sb = cstp.tile([128, S], bf, name="cos_sb", tag="cos_sb")
        sin_sb = cstp.tile([128, S], bf, name="sin_sb", tag="sin_sb")

        resid = stack.enter_context(tc.tile_pool(name="resid", bufs=1))
        kT = [resid.tile([128, S], bf, name=f"kT{i}", tag=f"kT{i}") for i in range(8)]
        qT = [resid.tile([128, T], bf, name=f"qT{i}", tag=f"qT{i}") for i in range(8)]
        v_sb = [resid.tile([128, DIN], bf, name=f"v{j}", tag=f"v{j}")
                for j in range(16)]
        yT = [resid.tile([128, T], bf, name=f"yT{i}", tag=f"yT{i}") for i in range(8)]
        wo_sb = [resid.tile([128, DOUT], bf, name=f"wo{i}", tag=f"wo{i}")
                 for i in range(8)]

        # ---------------- Phase A: projections + rope ----------------
        with tc.tile_pool(name="paxT", bufs=1) as xtp, \
             tc.tile_pool(name="pawv", bufs=1) as wvp, \
             tc.tile_pool(name="pawt", bufs=3) as wtp, \
             tc.tile_pool(name="papk", bufs=2) as pkp, \
             tc.tile_pool(name="patmp", bufs=4) as tmp, \
             tc.tile_pool(name="paraw", bufs=4) as rawp, \
             tc.tile_pool(name="paps", bufs=4, space="PSUM") as psp:
            xT = [xtp.tile([128, T], bf, name=f"xT{i}", tag=f"xT{i}")
                  for i in range(8)]
            for i in range(8):
                nc.sync.dma_start(out=xT[i][:], in_=xT_d[128 * i:128 * (i + 1), :])
            nc.sync.dma_start(out=cos_sb[:], in_=cos_d[:])
            nc.sync.dma_start(out=sin_sb[:], in_=sin_d[:])
            nc.sync.dma_start(out=masks[:], in_=masks_d[:])
            nc.sync.dma_start(out=ones_sb[:], in_=ones_d[:])
            nc.sync.dma_start(out=onesr_sb[:], in_=onesr_d[:])

            # q/k projections per head; rope applied on the PSUM outputs.
            # Even f-tile (2h) and odd (2h+1) rows share the same cos/sin rows.
            for w_d, dst, off in ((wq_d, qT, 0), (wk_d, kT, P)):
                for h in range(4):
                    psl = [psp.tile([128, 512], f32, name=f"pj{i}", tag=f"pj{i}",
                                    bufs=1) for i in range(4)]
                    for kt in range(8):
                        wt = wtp.tile([128, 256], bf, name="wload", tag="wload")
                        nc.sync.dma_start(
                            out=wt[:],
                            in_=w_d[128 * kt:128 * (kt + 1), 256 * h:256 * (h + 1)])
                        for f2 in range(2):
                            for th in range(2):
                                nc.tensor.matmul(
                                    psl[2 * f2 + th][:],
                                    wt[:, 128 * f2:128 * (f2 + 1)],
                                    xT[kt][:, 512 * th:512 * (th + 1)],
                                    start=(kt == 0), stop=(kt == 7))
                    for th in range(2):
                        e_ps, o_ps = psl[th][:], psl[2 + th][:]
                        c = cos_sb[:, P + 512 * th:P + 512 * (th + 1)]
                        s = sin_sb[:, P + 512 * th:P + 512 * (th + 1)]
                        e_sb = rawp.tile([128, 512], bf, name="e_sb", tag="e_sb")
                        o_sb = rawp.tile([128, 512], bf, name="o_sb", tag="o_sb")
                        nc.scalar.copy(e_sb[:], e_ps)
                        nc.scalar.copy(o_sb[:], o_ps)
                        t1 = tmp.tile([128, 512], bf, name="t1", tag="t1")
                        t2 = tmp.tile([128, 512], bf, name="t2", tag="t2")
                        t3 = tmp.tile([128, 512], bf, name="t3", tag="t3")
                        t4 = tmp.tile([128, 512], bf, name="t4", tag="t4")
                        nc.vector.tensor_tensor(t1[:], e_sb[:], c, op=OP.mult)
                        nc.gpsimd.tensor_tensor(t2[:], o_sb[:], s, op=OP.mult)
                        nc.vector.tensor_tensor(t3[:], o_sb[:], c, op=OP.mult)
                        nc.gpsimd.tensor_tensor(t4[:], e_sb[:], s, op=OP.mult)
                        de = dst[2 * h][:, off + 512 * th:off + 512 * (th + 1)]
                        do = dst[2 * h + 1][:, off + 512 * th:off + 512 * (th + 1)]
                        nc.vector.tensor_tensor(de, t1[:], t2[:], op=OP.subtract)
                        nc.vector.tensor_tensor(do, t3[:], t4[:], op=OP.add)

            # past_k rope: DMA f-major tiles, combine into kT[:, 0:P]
            for h in range(4):
                pe = pkp.tile([128, P], bf, name="pke", tag="pke")
                po = pkp.tile([128, P], bf, name="pko", tag="pko")
                nc.sync.dma_start(out=pe[:], in_=pkT_d[2 * h])
                nc.sync.dma_start(out=po[:], in_=pkT_d[2 * h + 1])
                for sh in range(2):
                    sl = slice(512 * sh, 512 * (sh + 1))
                    c, s = cos_sb[:, sl], sin_sb[:, sl]
                    t1 = tmp.tile([128, 512], bf, name="t1", tag="t1")
                    t2 = tmp.tile([128, 512], bf, name="t2", tag="t2")
                    t3 = tmp.tile([128, 512], bf, name="t3", tag="t3")
                    t4 = tmp.tile([128, 512], bf, name="t4", tag="t4")
                    nc.gpsimd.tensor_tensor(t1[:], pe[:, sl], c, op=OP.mult)
                    nc.gpsimd.tensor_tensor(t2[:], po[:, sl], s, op=OP.mult)
                    nc.gpsimd.tensor_tensor(t3[:], po[:, sl], c, op=OP.mult)
                    nc.gpsimd.tensor_tensor(t4[:], pe[:, sl], s, op=OP.mult)
                    nc.vector.tensor_tensor(kT[2 * h][:, sl], t1[:], t2[:],
                                            op=OP.subtract)
                    nc.vector.tensor_tensor(kT[2 * h + 1][:, sl], t3[:], t4[:],
                                            op=OP.add)

            # past_v straight in (natural [s, h*f] layout)
            for j in range(8):
                nc.sync.dma_start(out=v_sb[j][:],
                                  in_=pv_d[128 * j:128 * (j + 1), :])

            # v projection, natural layout [s, f]
            wv_sb = [wvp.tile([128, DIN], bf, name=f"wv{i}", tag=f"wv{i}")
                     for i in range(8)]
            for kt in range(8):
                nc.sync.dma_start(out=wv_sb[kt][:],
                                  in_=wv_d[128 * kt:128 * (kt + 1), :])
            for stg in range(4):
                psl = [psp.tile([128, 512], f32, name=f"pv{i}", tag=f"pj{i}",
                                bufs=1) for i in range(4)]
                for kt in range(8):
                    for s2 in range(2):
                        st = 2 * stg + s2
                        for fh in range(2):
                            nc.tensor.matmul(
                                psl[2 * s2 + fh][:],
                                xT[kt][:, 128 * st:128 * (st + 1)],
                                wv_sb[kt][:, 512 * fh:512 * (fh + 1)],
                                start=(kt == 0), stop=(kt == 7))
                for s2 in range(2):
                    st = 2 * stg + s2
                    for fh in range(2):
                        nc.scalar.copy(v_sb[8 + st][:, 512 * fh:512 * (fh + 1)],
                                       psl[2 * s2 + fh][:])

            # wo for phase C
            for kt in range(8):
                nc.sync.dma_start(out=wo_sb[kt][:],
                                  in_=wo_d[128 * kt:128 * (kt + 1), :])

        # ---------------- Phase B+C: attention + o-projection ----------------
        with tc.tile_pool(name="probs", bufs=5) as prp, \
             tc.tile_pool(name="pbsm", bufs=2) as smp, \
             tc.tile_pool(name="pbo", bufs=2) as osp, \
             tc.tile_pool(name="pbsc", bufs=2, space="PSUM") as scps, \
             tc.tile_pool(name="pby", bufs=1, space="PSUM") as yps:
            for TH in range(2):
                for h in range(4):
                    jmax = 12 + 4 * TH
                    ytp_ps = [yps.tile([128, 512], f32, name=f"ytp{i}",
                                       tag=f"ytp{i}", bufs=1) for i in range(2)]
                    sm_ps = yps.tile([1, 512], f32, name="smps", tag="smps",
                                     bufs=1)
                    for j in range(jmax):
                        sc = scps.tile([128, 512], f32, name="sc", tag="sc")
                        for fk in range(2):
                            nc.tensor.matmul(
                                sc[:],
                                kT[2 * h + fk][:, 128 * j:128 * (j + 1)],
                                qT[2 * h + fk][:, 512 * TH:512 * (TH + 1)],
                                start=(fk == 0), stop=(fk == 1))
                        pj = prp.tile([128, 512], bf, name="pj", tag="pj")
                        nc.scalar.activation(pj[:], sc[:], AF.Exp,
                                             scale=float(DH ** -0.5))
                        ci = j - (8 + 4 * TH)
                        if ci >= 0:
                            nc.vector.tensor_tensor(pj[:], pj[:], masks[:, ci, :],
                                                    op=OP.mult)
                        for fb in range(2):
                            nc.tensor.matmul(
                                ytp_ps[fb][:],
                                v_sb[j][:, 256 * h + 128 * fb:256 * h + 128 * (fb + 1)],
                                pj[:],
                                start=(j == 0), stop=(j == jmax - 1))
                        nc.tensor.matmul(
                            sm_ps[:], ones_sb[:], pj[:],
                            start=(j == 0), stop=(j == jmax - 1))
                    srow = smp.tile([1, 512], f32r, name="srow", tag="srow")
                    nc.scalar.copy(srow[:], sm_ps[:])
                    bc_ps = scps.tile([128, 512], f32, name="bcps", tag="bcps",
                                      bufs=1)
                    nc.tensor.matmul(bc_ps[:], onesr_sb[:], srow[:],
                                     start=True, stop=True)
                    rbc = smp.tile([128, 512], f32, name="rbc", tag="rbc")
                    nc.vector.reciprocal_approx_fast(out=rbc[:], in_=bc_ps[:])
                    for fb in range(2):
                        nc.vector.tensor_tensor(
                            yT[2 * h + fb][:, 512 * TH:512 * (TH + 1)],
                            ytp_ps[fb][:],
                            rbc[:],
                            op=OP.mult)
                # o-projection for this TH's four t-tiles
                for tt in range(4 * TH, 4 * TH + 4):
                    ot = osp.tile([128, DOUT], f32, name="osb", tag="osb")
                    for ds in range(3):
                        op_ps = scps.tile([128, 384], f32, name="ops", tag="ops",
                                          bufs=2)
                        for fk in range(8):
                            nc.tensor.matmul(
                                op_ps[:],
                                yT[fk][:, 128 * tt:128 * (tt + 1)],
                                wo_sb[fk][:, 384 * ds:384 * (ds + 1)],
                                start=(fk == 0), stop=(fk == 7))
                        nc.scalar.copy(ot[:, 384 * ds:384 * (ds + 1)], op_ps[:])
                    nc.sync.dma_start(out=out_d[128 * tt:128 * (tt + 1), :],
                                      in_=ot[:])

    nc.finalize()
    return nc


_NC_CACHE = {}


def _prep_inputs(x, past_k, past_v, wq, wk, wv, wo):
    bf16 = _bf16()
    consts = _host_constants()
    perm = _qk_perm()
    wq_p = np.ascontiguousarray(wq[:, perm]).astype(bf16)
    wk_p = np.ascontiguousarray(wk[:, perm]).astype(bf16)
    wv_b = wv.astype(bf16)
    wo_b = wo.astype(bf16)
    in_maps = []
    for b in range(NCORES):
        pk = past_k[b]  # [P, H, DH]
        pkp = np.concatenate([pk[:, :, 0::2], pk[:, :, 1::2]], axis=2)  # [P,H,256]
        pkT = np.ascontiguousarray(pkp.transpose(1, 2, 0)).reshape(8, 128, P)
        m = {
            "xT": np.ascontiguousarray(x[b].T).astype(bf16),
            "pkT": pkT.astype(bf16),
            "pv": np.ascontiguousarray(past_v[b].reshape(P, DIN)).astype(bf16),
            "wq": wq_p, "wk": wk_p, "wv": wv_b, "wo": wo_b,
            "cos": consts["cos"], "sin": consts["sin"],
            "masks": consts["masks"], "ones": consts["ones"],
            "onesr": consts["onesr"],
        }
        in_maps.append(m)
    return in_maps


def run(x, past_k, past_v, wq, wk, wv, wo, trace=False):
    from concourse.bass_utils import run_bass_kernel_spmd

    if "nc" not in _NC_CACHE:
        _NC_CACHE["nc"] = build_kernel()
    nc = _NC_CACHE["nc"]
    in_maps = _prep_inputs(x, past_k, past_v, wq, wk, wv, wo)
    res = run_bass_kernel_spmd(nc, in_maps, list(range(NCORES)), trace=trace)
    out = np.stack([res.results[b]["out"] for b in range(NCORES)], axis=0)
    return out, res


def kernel(x, past_k, past_v, wq, wk, wv, wo):
    out, _ = run(x, past_k, past_v, wq, wk, wv, wo)
    return out


# revision 5
# speedup vs baseline: 1.6226x; 1.0676x over previous
"""Bass/Trainium2 kernel for nn_BakaAttention: 8-way data-parallel over batch.

Per core (one batch element):
  q = rope(x@wq, off=1024); k = rope(concat(past_k, x@wk), off=0); v = concat(past_v, x@wv)
  out = softmax(mask(q k^T / 16)) v @ wo

All matmul operands are bf16 (1 cycle/row on PE, half the DMA/SBUF bytes);
PSUM accumulation stays f32. The host pre-arranges every input into its exact
SBUF tile layout (x transposed, past_k feature-major, weights tiled) so each
tensor lands in one contiguous DMA, and pre-permutes q/k feature order per head
to [even feats | odd feats], which turns interleaved-pair rope into
partition-aligned elementwise ops (no on-chip transposes or rotation matmuls).
Scores are computed transposed [s, t] so PV consumes probs directly as the
moving operand; everything stays SBUF-resident (no DRAM round trips).
"""

import numpy as np

B, T, P, H, DH, DIN, DOUT = 8, 1024, 1024, 4, 256, 1024, 1152
S = P + T  # 2048 keys
THETA = 10000.0
NCORES = 8


def _bf16():
    import ml_dtypes
    return ml_dtypes.bfloat16


def _host_constants():
    bf16 = _bf16()
    m = np.arange(128, dtype=np.float64)
    inv = 1.0 / (THETA ** (m / 128.0))                      # [128]
    pos = np.arange(S, dtype=np.float64)                    # [2048]
    ang = np.outer(inv, pos)                                # [128, 2048]
    consts = {
        "cos": np.cos(ang).astype(bf16),
        "sin": np.sin(ang).astype(bf16),
    }
    # masks[sl, ci, tl] = 1.0 if sl <= tl - 128*ci else 0 (keep), ci in 0..3
    sl = np.arange(128)[:, None]
    tl = np.arange(512)[None, :]
    masks = np.stack(
        [(sl <= tl - 128 * ci).astype(bf16) for ci in range(4)], axis=1
    )  # [128, 4, 512]
    consts["masks"] = np.ascontiguousarray(masks)
    consts["ones"] = np.ones((128, 1), bf16)
    consts["onesr"] = np.ones((1, 128), np.float32)
    return consts


# column permutation putting each head's even features first, then odd
def _qk_perm():
    idx = np.arange(DIN).reshape(H, DH // 2, 2)
    return np.concatenate([idx[..., 0], idx[..., 1]], axis=1).reshape(-1)


def build_kernel():
    import concourse.bass as bass
    import concourse.mybir as mybir
    from concourse import bacc
    from concourse.tile import TileContext

    f32 = mybir.dt.float32
    f32r = mybir.dt.float32r
    bf = mybir.dt.bfloat16
    AF = mybir.ActivationFunctionType
    OP = mybir.AluOpType

    nc = bacc.Bacc(None, target_bir_lowering=False)

    # host-prearranged layouts: leading dims match SBUF [partition, slab, col]
    xT_d = nc.dram_tensor("xT", [128, 8, T], bf, kind="ExternalInput")
    pkT_d = nc.dram_tensor("pkT", [128, 8, P], bf, kind="ExternalInput")
    pv_d = nc.dram_tensor("pv", [128, 8, DIN], bf, kind="ExternalInput")
    wq_d = nc.dram_tensor("wq", [4, 128, 8, 256], bf, kind="ExternalInput")
    wk_d = nc.dram_tensor("wk", [4, 128, 8, 256], bf, kind="ExternalInput")
    wv_d = nc.dram_tensor("wv", [128, 8, DIN], bf, kind="ExternalInput")
    wo_d = nc.dram_tensor("wo", [128, 8, DOUT], bf, kind="ExternalInput")
    cos_d = nc.dram_tensor("cos", [128, S], bf, kind="ExternalInput")
    sin_d = nc.dram_tensor("sin", [128, S], bf, kind="ExternalInput")
    masks_d = nc.dram_tensor("masks", [128, 4, 512], bf, kind="ExternalInput")
    ones_d = nc.dram_tensor("ones", [128, 1], bf, kind="ExternalInput")
    onesr_d = nc.dram_tensor("onesr", [1, 128], f32r, kind="ExternalInput")
    out_d = nc.dram_tensor("out", [T, DOUT], f32, kind="ExternalOutput")

    from contextlib import ExitStack
    stack = ExitStack()
    with TileContext(nc) as tc, stack:
        cstp = stack.enter_context(tc.tile_pool(name="consts", bufs=1))
        masks = cstp.tile([128, 4, 512], bf, name="masks", tag="masks")
        ones_sb = cstp.tile([128, 1], bf, name="ones_sb", tag="ones_sb")
        onesr_sb = cstp.tile([1, 128], f32r, name="onesr_sb", tag="onesr_sb")
        cos_sb = cstp.tile([128, S], bf, name="cos_sb", tag="cos_sb")
        sin_sb = cstp.tile([128, S], bf, name="sin_sb", tag="sin_sb")

        resid = stack.enter_context(tc.tile_pool(name="resid", bufs=1))
        kT = [resid.tile([128, S], bf, name=f"kT{i}", tag=f"kT{i}") for i in range(8)]
        qT = [resid.tile([128, T], bf, name=f"qT{i}", tag=f"qT{i}") for i in range(8)]
        vbig = resid.tile([128, 16, DIN], bf, name="vbig", tag="vbig")
        yT = [resid.tile([128, T], bf, name=f"yT{i}", tag=f"yT{i}") for i in range(8)]

        # ---------------- Phase A: projections + rope ----------------
        with tc.tile_pool(name="paxT", bufs=1) as xtp, \
             tc.tile_pool(name="pawv", bufs=1) as wvp, \
             tc.tile_pool(name="papk", bufs=1) as pkp, \
             tc.tile_pool(name="pawt", bufs=2) as wtp, \
             tc.tile_pool(name="patmp", bufs=2) as tmp, \
             tc.tile_pool(name="paraw", bufs=2) as rawp, \
             tc.tile_pool(name="paps", bufs=4, space="PSUM") as psp:
            # startup-critical DMAs first (x halves, tables, past_k)
            xbig = xtp.tile([128, 8, T], bf, name="xbig", tag="xbig")
            nc.sync.dma_start(out=xbig[:, 0:4, :], in_=xT_d[:, 0:4, :])
            nc.sync.dma_start(out=xbig[:, 4:8, :], in_=xT_d[:, 4:8, :])
            nc.sync.dma_start(out=cos_sb[:], in_=cos_d[:])
            nc.sync.dma_start(out=sin_sb[:], in_=sin_d[:])
            pkbig = pkp.tile([128, 8, P], bf, name="pkbig", tag="pkbig")
            nc.sync.dma_start(out=pkbig[:], in_=pkT_d[:])
            nc.sync.dma_start(out=masks[:], in_=masks_d[:])
            nc.sync.dma_start(out=ones_sb[:], in_=ones_d[:])
            nc.sync.dma_start(out=onesr_sb[:], in_=onesr_d[:])

            # past_k rope: combine directly from the DMA'd f-major tiles into
            # kT[:, 0:P]; pure vector/gpsimd work that runs under the PE
            # projection phase.
            for h in range(4):
                for sh in range(2):
                    sl = slice(512 * sh, 512 * (sh + 1))
                    c, s = cos_sb[:, sl], sin_sb[:, sl]
                    pe, po = pkbig[:, 2 * h, sl], pkbig[:, 2 * h + 1, sl]
                    t1 = tmp.tile([128, 512], bf, name="t1", tag="t1")
                    t2 = tmp.tile([128, 512], bf, name="t2", tag="t2")
                    t3 = tmp.tile([128, 512], bf, name="t3", tag="t3")
                    t4 = tmp.tile([128, 512], bf, name="t4", tag="t4")
                    nc.vector.tensor_tensor(t1[:], pe, c, op=OP.mult)
                    nc.gpsimd.tensor_tensor(t2[:], po, s, op=OP.mult)
                    nc.vector.tensor_tensor(t3[:], po, c, op=OP.mult)
                    nc.gpsimd.tensor_tensor(t4[:], pe, s, op=OP.mult)
                    nc.vector.tensor_tensor(kT[2 * h][:, sl], t1[:], t2[:],
                                            op=OP.subtract)
                    nc.vector.tensor_tensor(kT[2 * h + 1][:, sl], t3[:], t4[:],
                                            op=OP.add)

            # q/k projections per head; rope applied on the PSUM outputs.
            # Even f-tile (2h) and odd (2h+1) rows share the same cos/sin rows.
            for w_d, dst, off in ((wq_d, qT, 0), (wk_d, kT, P)):
                for h in range(4):
                    wth = wtp.tile([128, 8, 256], bf, name="wload", tag="wload")
                    nc.sync.dma_start(out=wth[:], in_=w_d[h])
                    psl = [psp.tile([128, 512], f32, name=f"pj{i}", tag=f"pj{i}",
                                    bufs=1) for i in range(4)]
                    for kt in range(8):
                        for f2 in range(2):
                            for th in range(2):
                                nc.tensor.matmul(
                                    psl[2 * f2 + th][:],
                                    wth[:, kt, 128 * f2:128 * (f2 + 1)],
                                    xbig[:, kt, 512 * th:512 * (th + 1)],
                                    start=(kt == 0), stop=(kt == 7))
                    for th in range(2):
                        e_ps, o_ps = psl[th][:], psl[2 + th][:]
                        c = cos_sb[:, P + 512 * th:P + 512 * (th + 1)]
                        s = sin_sb[:, P + 512 * th:P + 512 * (th + 1)]
                        e_sb = rawp.tile([128, 512], bf, name="e_sb", tag="e_sb")
                        o_sb = rawp.tile([128, 512], bf, name="o_sb", tag="o_sb")
                        nc.scalar.copy(e_sb[:], e_ps)
                        nc.scalar.copy(o_sb[:], o_ps)
                        t1 = tmp.tile([128, 512], bf, name="t1", tag="t1")
                        t2 = tmp.tile([128, 512], bf, name="t2", tag="t2")
                        t3 = tmp.tile([128, 512], bf, name="t3", tag="t3")
                        t4 = tmp.tile([128, 512], bf, name="t4", tag="t4")
                        nc.vector.tensor_tensor(t1[:], e_sb[:], c, op=OP.mult)
                        nc.gpsimd.tensor_tensor(t2[:], o_sb[:], s, op=OP.mult)
                        nc.vector.tensor_tensor(t3[:], o_sb[:], c, op=OP.mult)
                        nc.gpsimd.tensor_tensor(t4[:], e_sb[:], s, op=OP.mult)
                        de = dst[2 * h][:, off + 512 * th:off + 512 * (th + 1)]
                        do = dst[2 * h + 1][:, off + 512 * th:off + 512 * (th + 1)]
                        nc.vector.tensor_tensor(de, t1[:], t2[:], op=OP.subtract)
                        nc.vector.tensor_tensor(do, t3[:], t4[:], op=OP.add)

            # past_v straight in (natural [s, h*f] layout)
            nc.sync.dma_start(out=vbig[:, 0:8, :], in_=pv_d[:])

            # v projection, natural layout [s, f]
            wvbig = wvp.tile([128, 8, DIN], bf, name="wvbig", tag="wvbig")
            nc.sync.dma_start(out=wvbig[:], in_=wv_d[:])
            for stg in range(4):
                psl = [psp.tile([128, 512], f32, name=f"pv{i}", tag=f"pj{i}",
                                bufs=1) for i in range(4)]
                for kt in range(8):
                    for s2 in range(2):
                        st = 2 * stg + s2
                        for fh in range(2):
                            nc.tensor.matmul(
                                psl[2 * s2 + fh][:],
                                xbig[:, kt, 128 * st:128 * (st + 1)],
                                wvbig[:, kt, 512 * fh:512 * (fh + 1)],
                                start=(kt == 0), stop=(kt == 7))
                for s2 in range(2):
                    st = 2 * stg + s2
                    for fh in range(2):
                        nc.scalar.copy(vbig[:, 8 + st, 512 * fh:512 * (fh + 1)],
                                       psl[2 * s2 + fh][:])

        # ---------------- Phase B+C: attention + o-projection ----------------
        with tc.tile_pool(name="pbwo", bufs=1) as wop, \
             tc.tile_pool(name="probs", bufs=5) as prp, \
             tc.tile_pool(name="pbsm", bufs=2) as smp, \
             tc.tile_pool(name="pbo", bufs=2) as osp, \
             tc.tile_pool(name="pbsc", bufs=2, space="PSUM") as scps, \
             tc.tile_pool(name="pby", bufs=1, space="PSUM") as yps:
            wobig = wop.tile([128, 8, DOUT], bf, name="wobig", tag="wobig")
            nc.sync.dma_start(out=wobig[:], in_=wo_d[:])
            for TH in (1, 0):
                for h in range(4):
                    jmax = 12 + 4 * TH
                    ytp_ps = [yps.tile([128, 512], f32, name=f"ytp{i}",
                                       tag=f"ytp{i}", bufs=1) for i in range(2)]
                    sm_ps = yps.tile([1, 512], f32, name="smps", tag="smps",
                                     bufs=1)
                    for j in range(jmax):
                        sc = scps.tile([128, 512], f32, name="sc", tag="sc")
                        for fk in range(2):
                            nc.tensor.matmul(
                                sc[:],
                                kT[2 * h + fk][:, 128 * j:128 * (j + 1)],
                                qT[2 * h + fk][:, 512 * TH:512 * (TH + 1)],
                                start=(fk == 0), stop=(fk == 1))
                        pj = prp.tile([128, 512], bf, name="pj", tag="pj")
                        nc.scalar.activation(pj[:], sc[:], AF.Exp,
                                             scale=float(DH ** -0.5))
                        ci = j - (8 + 4 * TH)
                        if ci >= 0:
                            nc.vector.tensor_tensor(pj[:], pj[:], masks[:, ci, :],
                                                    op=OP.mult)
                        for fb in range(2):
                            nc.tensor.matmul(
                                ytp_ps[fb][:],
                                vbig[:, j, 256 * h + 128 * fb:256 * h + 128 * (fb + 1)],
                                pj[:],
                                start=(j == 0), stop=(j == jmax - 1))
                        nc.tensor.matmul(
                            sm_ps[:], ones_sb[:], pj[:],
                            start=(j == 0), stop=(j == jmax - 1))
                    srow = smp.tile([1, 512], f32r, name="srow", tag="srow")
                    nc.scalar.copy(srow[:], sm_ps[:])
                    bc_ps = scps.tile([128, 512], f32, name="bcps", tag="bcps",
                                      bufs=1)
                    nc.tensor.matmul(bc_ps[:], onesr_sb[:], srow[:],
                                     start=True, stop=True)
                    rbc = smp.tile([128, 512], f32, name="rbc", tag="rbc")
                    nc.vector.reciprocal_approx_fast(out=rbc[:], in_=bc_ps[:])
                    for fb in range(2):
                        nc.vector.tensor_tensor(
                            yT[2 * h + fb][:, 512 * TH:512 * (TH + 1)],
                            ytp_ps[fb][:],
                            rbc[:],
                            op=OP.mult)
                # o-projection for this TH's four t-tiles
                for tt in range(4 * TH, 4 * TH + 4):
                    ot = osp.tile([128, DOUT], f32, name="osb", tag="osb")
                    for ds in range(3):
                        op_ps = scps.tile([128, 384], f32, name="ops", tag="ops",
                                          bufs=2)
                        for fk in range(8):
                            nc.tensor.matmul(
                                op_ps[:],
                                yT[fk][:, 128 * tt:128 * (tt + 1)],
                                wobig[:, fk, 384 * ds:384 * (ds + 1)],
                                start=(fk == 0), stop=(fk == 7))
                        nc.scalar.copy(ot[:, 384 * ds:384 * (ds + 1)], op_ps[:])
                    nc.sync.dma_start(out=out_d[128 * tt:128 * (tt + 1), :],
                                      in_=ot[:])

    nc.finalize()
    return nc


_NC_CACHE = {}


def _prep_inputs(x, past_k, past_v, wq, wk, wv, wo):
    bf16 = _bf16()
    consts = _host_constants()
    perm = _qk_perm()

    def wtile(w):  # [1024, 1024] -> [4 head, 128 part, 8 kt, 256 col]
        return np.ascontiguousarray(
            w.reshape(8, 128, 4, 256).transpose(2, 1, 0, 3)).astype(bf16)

    def ptile(a):  # [1024, C] -> [128 part, 8 slab, C]
        c = a.shape[-1]
        return np.ascontiguousarray(
            a.reshape(8, 128, c).transpose(1, 0, 2)).astype(bf16)

    wq_p = wtile(wq[:, perm])
    wk_p = wtile(wk[:, perm])
    wv_b = ptile(wv)
    wo_b = ptile(wo)
    in_maps = []
    for b in range(NCORES):
        pk = past_k[b]  # [P, H, DH]
        pkp = np.concatenate([pk[:, :, 0::2], pk[:, :, 1::2]], axis=2)  # [P,H,256]
        pkT = np.ascontiguousarray(pkp.transpose(1, 2, 0).reshape(8, 128, P)
                                   .transpose(1, 0, 2)).astype(bf16)
        m = {
            "xT": ptile(np.ascontiguousarray(x[b].T)),
            "pkT": pkT,
            "pv": ptile(past_v[b].reshape(P, DIN)),
            "wq": wq_p, "wk": wk_p, "wv": wv_b, "wo": wo_b,
            "cos": consts["cos"], "sin": consts["sin"],
            "masks": consts["masks"], "ones": consts["ones"],
            "onesr": consts["onesr"],
        }
        in_maps.append(m)
    return in_maps


def run(x, past_k, past_v, wq, wk, wv, wo, trace=False):
    from concourse.bass_utils import run_bass_kernel_spmd

    if "nc" not in _NC_CACHE:
        _NC_CACHE["nc"] = build_kernel()
    nc = _NC_CACHE["nc"]
    in_maps = _prep_inputs(x, past_k, past_v, wq, wk, wv, wo)
    res = run_bass_kernel_spmd(nc, in_maps, list(range(NCORES)), trace=trace)
    out = np.stack([res.results[b]["out"] for b in range(NCORES)], axis=0)
    return out, res


def kernel(x, past_k, past_v, wq, wk, wv, wo):
    out, _ = run(x, past_k, past_v, wq, wk, wv, wo)
    return out
